# revision 15
# baseline (speedup 1.0000x reference)
"""Set-Transformer encoder (2x SAB sigmoid-attention + PMA) on 8 TRN2 cores.

Sharding: core c handles batch b=c//2, query-half hf=c%2 (1024 of 2048 rows).
All data flows feature-major ([D=128 partitions, tokens]); the host supplies
X pre-transposed and pre-cast to bf16.  Between SAB layers each core pair
AllGathers its half of the layer output in two query-chunks, launched as
soon as each chunk is ready so the exchange hides under the remaining
attention work; the next layer processes the keys covered by the first
chunk before the second arrives (attention is permutation-invariant over
keys).  The PMA + final projection are computed redundantly by both cores
of a pair.

Matmul operands are bf16 (1 cycle/row on PE); accumulation and the residual
spine stay fp32.  The per-head (dh=32) QK matmuls use 32x32 tile_position
packing (8 concurrent tiles per 128-key group); AV contracts the full 128
keys with col-banded (M=32) matmuls accumulating O^T in place.
"""
import numpy as np
import ml_dtypes

import concourse.bass as bass
import concourse.tile as tile
from concourse import mybir
from concourse.bass_utils import run_bass_kernel_spmd

B, N, D, H, DH, DOUT = 4, 2048, 128, 4, 32, 256
NQ = N // 2          # queries per core
QCH = 512            # query chunk (matmul moving-dim)
NKT = N // 128       # 16 key tiles
SCALE = 1.0 / np.sqrt(np.float32(D))  # 1/sqrt(128) logit scale

fp32 = mybir.dt.float32
bf16 = mybir.dt.bfloat16
ALU = mybir.AluOpType
SIG = mybir.ActivationFunctionType.Sigmoid
PAIRS = [[0, 1], [2, 3], [4, 5], [6, 7]]
DEBUG_TAPS = False

# key-tile processing order when keys arrive via 2-chunk AllGather:
# AG chunk a carries each core's queries [0:512) -> global keys
# [0:512) u [1024:1536) = key tiles 0-3 and 8-11.
KT_ORDER_AG = [0, 1, 2, 3, 8, 9, 10, 11, 4, 5, 6, 7, 12, 13, 14, 15]
KCH_ORDER_AG = [0, 2, 1, 3]          # 512-col projection chunk order


def _fix_excess_waits(nc):
    """walrus accepts very few sync waits per instruction; hoist excess
    waits onto preceding same-engine NOPs (same stream => same semantics)."""
    for f in nc.m.functions:
        for bb in f.blocks:
            new_list = []
            for ins in bb.instructions:
                si = ins.sync_info
                cap = 2 if isinstance(ins, mybir.InstEventSemaphore) else 1
                if si is not None and len(si.on_wait) > cap:
                    waits = list(si.on_wait)
                    excess, kept = waits[:-cap], waits[-cap:]
                    for j, w in enumerate(excess):
                        nop = mybir.InstNoOp(
                            name=f"{ins.name}-presync{j}", ins=[], outs=[]
                        )
                        nop.engine = ins.engine
                        nop.sync_info = mybir.SyncInfo(on_wait=[w], on_update=[])
                        nc.register_instruction(nop)
                        new_list.append(nop)
                    ins.sync_info = mybir.SyncInfo(
                        on_wait=kept, on_update=list(si.on_update)
                    )
                new_list.append(ins)
            bb.instructions = new_list


def _bcast(ap, n):
    return ap.to_broadcast([ap.shape[0], n])


def _load_weights(nc, sbuf, shapes):
    tiles = {}
    for key, (shape, dt) in shapes.items():
        p = nc.declare_dram_parameter(key, shape, dt if dt == bf16 else fp32,
                                      isOutput=False)
        t = sbuf.tile(shape, dt, tag=f"in_{key}")
        nc.gpsimd.dma_start(out=t[:], in_=p[:])
        tiles[key] = t
    return tiles


def _proj_q(nc, pools, XTq, w, i, tagp):
    sbuf, psP = pools["sbuf"], pools["psP"]
    Wq, bq = w[f"w{i}q"], w[f"b{i}q"]
    QTf = sbuf.tile([128, NQ], fp32, tag=f"{tagp}QTf")
    QTb = sbuf.tile([128, NQ], bf16, tag=f"{tagp}QTb")
    for c in range(2):
        ps = psP.tile([128, QCH], fp32, tag="proj")
        nc.tensor.matmul(ps[:], lhsT=Wq[:], rhs=XTq[:, c * QCH:(c + 1) * QCH],
                         start=True, stop=True)
        nc.vector.tensor_tensor(
            out=QTf[:, c * QCH:(c + 1) * QCH], in0=ps[:],
            in1=_bcast(bq[:, 0:1], QCH), op=ALU.add)
        nc.vector.tensor_tensor(
            out=QTb[:, c * QCH:(c + 1) * QCH], in0=ps[:],
            in1=_bcast(bq[:, 0:1], QCH), op=ALU.add)
    return QTf, QTb


def _proj_kv_wave(nc, pools, XTfull, w, i, KT, V, kchs, kts):
    """K^T chunks + V tiles for one wave of arrived keys."""
    psP = pools["psP"]
    Wk, Wv = w[f"w{i}k"], w[f"w{i}v"]
    bk, bvb = w[f"b{i}k"], w[f"b{i}v"]
    for c in kchs:
        ps = psP.tile([128, QCH], fp32, tag="proj")
        nc.tensor.matmul(ps[:], lhsT=Wk[:], rhs=XTfull[:, c * QCH:(c + 1) * QCH],
                         start=True, stop=True)
        nc.vector.tensor_tensor(
            out=KT[:, c * QCH:(c + 1) * QCH], in0=ps[:],
            in1=_bcast(bk[:, 0:1], QCH), op=ALU.add)
    for t in kts:
        ps = psP.tile([128, QCH], fp32, tag="proj")
        nc.tensor.matmul(ps[:, 0:128], lhsT=XTfull[:, t * 128:(t + 1) * 128],
                         rhs=Wv[:], start=True, stop=True)
        nc.vector.tensor_tensor(out=V[:, t * 128:(t + 1) * 128], in0=ps[:, 0:128],
                                in1=bvb[:], op=ALU.add)


def _sab(nc, pools, XTfull, XTq, w, i, tagp, waves, emit_ag, post_qc=None):
    """One SAB layer; returns (XhT_half, XTnext or None).

    waves: list of (kch_list, kt_list) -- keys grouped by arrival order.
    post_qc: optional {qc: fn(XhT, qc)} called after each query chunk."""
    sbuf, sbufA, psL, psO, psF, dram = (
        pools["sbuf"], pools["sbufA"], pools["psL"], pools["psO"], pools["psF"],
        pools["dram"],
    )
    Wo, bo = w[f"w{i}o"], w[f"b{i}o"]
    kt_order = [t for _, kts in waves for t in kts]
    QTf, QTb = _proj_q(nc, pools, XTq, w, i, tagp)
    KT = sbuf.tile([128, N], bf16, tag=f"{tagp}KT")
    V = sbuf.tile([128, N], bf16, tag=f"{tagp}V")

    XhT = sbuf.tile([128, NQ], bf16, tag=f"{tagp}XhT")
    OTf = sbuf.tile([128, NQ], fp32, tag=f"{tagp}OTf")
    OTb = sbuf.tile([128, NQ], bf16, tag=f"{tagp}OTb")
    XTnext = None
    if emit_ag:
        XTnext = sbuf.tile([128, N], bf16, tag=f"{tagp}XTn")

    for qc in range(2):
        qs = qc * QCH
        OTps = psO.tile([128, QCH], fp32, tag="OT")
        groups = [(kt, hp) for kt in kt_order for hp in range(2)]
        Ltiles = {}
        # wave w's projections are emitted just before its first group
        # (first qc pass only); the scheduler starts them as keys arrive.
        proj_at = {}
        if qc == 0:
            gidx = 0
            for kchs, kts in waves:
                proj_at[gidx] = (kchs, kts)
                gidx += 2 * len(kts)

        def emit_qk(g):
            kt, hp = groups[g]
            L = psL.tile([128, 1024], fp32, tag="L")
            Ltiles[g] = L
            for h in (2 * hp, 2 * hp + 1):
                for j in range(4):
                    nc.tensor.matmul(
                        out=L[32 * j:32 * j + 32,
                              QCH * (h - 2 * hp):QCH * (h - 2 * hp) + QCH],
                        lhsT=KT[32 * h:32 * h + 32,
                                128 * kt + 32 * j:128 * kt + 32 * j + 32],
                        rhs=QTb[32 * h:32 * h + 32, qs:qs + QCH],
                        start=True, stop=True,
                        tile_position=(32 * h, 32 * j))

        ng = len(groups)
        pending_v = []

        def maybe_wave(gi):
            if gi in proj_at:
                kchs, kts = proj_at[gi]
                _proj_kv_wave(nc, pools, XTfull, w, i, KT, V, kchs, [])
                pending_v.append(kts)

        maybe_wave(0)
        emit_qk(0)
        if pending_v:
            _proj_kv_wave(nc, pools, XTfull, w, i, KT, V, [], pending_v.pop())
        for g in range(ng):
            kt, hp = groups[g]
            maybe_wave(g + 1)
            if g + 1 < ng:
                emit_qk(g + 1)
            if pending_v:
                _proj_kv_wave(nc, pools, XTfull, w, i, KT, V, [],
                              pending_v.pop())
            A = sbufA.tile([128, 1024], bf16, tag="A")
            nc.scalar.activation(A[:], Ltiles.pop(g)[:], SIG, scale=float(SCALE))
            # AV: A holds the full 128 keys of tile kt on partitions;
            # contract K=128 with one col-banded matmul per head.
            for h in (2 * hp, 2 * hp + 1):
                nc.tensor.matmul(
                    out=OTps[32 * h:32 * h + 32, 0:QCH],
                    lhsT=V[:, 128 * kt + 32 * h:128 * kt + 32 * h + 32],
                    rhs=A[:, QCH * (h - 2 * hp):QCH * (h - 2 * hp) + QCH],
                    start=(g // 2 == 0), stop=(g // 2 == NKT - 1),
                    tile_position=(0, 32 * h),
                    skip_group_check=True)

        # O = Qp + A@V ; Xh = O + relu(O @ Wo + bo)
        nc.vector.tensor_tensor(out=OTf[:, qs:qs + QCH], in0=OTps[:],
                                in1=QTf[:, qs:qs + QCH], op=ALU.add)
        nc.vector.tensor_tensor(out=OTb[:, qs:qs + QCH], in0=OTps[:],
                                in1=QTf[:, qs:qs + QCH], op=ALU.add)
        FC = psF.tile([128, QCH], fp32, tag="F")
        nc.tensor.matmul(FC[:], lhsT=Wo[:], rhs=OTb[:, qs:qs + QCH],
                         start=True, stop=True)
        R = sbuf.tile([128, QCH], fp32, tag="R")
        nc.vector.tensor_scalar(out=R[:], in0=FC[:], scalar1=bo[:, 0:1],
                                scalar2=0.0, op0=ALU.add, op1=ALU.max)
        nc.vector.tensor_tensor(out=XhT[:, qs:qs + QCH], in0=OTf[:, qs:qs + QCH],
                                in1=R[:], op=ALU.add)

        if emit_ag:
            # exchange this query chunk with the pair core right away
            cc_in = dram.tile([128, QCH], bf16, tag=f"{tagp}cci{qc}")
            nc.sync.dma_start(out=cc_in[:], in_=XhT[:, qs:qs + QCH])
            cc_out = dram.tile([256, QCH], bf16, tag=f"{tagp}cco{qc}")
            nc.gpsimd.collective_compute(
                "AllGather", ALU.bypass, replica_groups=PAIRS,
                ins=[cc_in[:]], outs=[cc_out[:]])
            # global columns: rank0 rows -> [qs:qs+512), rank1 -> [1024+qs:...)
            nc.sync.dma_start(out=XTnext[:, qs:qs + QCH], in_=cc_out[0:128, :])
            nc.sync.dma_start(out=XTnext[:, NQ + qs:NQ + qs + QCH],
                              in_=cc_out[128:256, :])
        if post_qc and qc in post_qc:
            post_qc[qc](XhT, qc)

    if DEBUG_TAPS:
        for nm, t in ((f"d{i}KT", KT), (f"d{i}QTb", QTb), (f"d{i}QTf", QTf),
                      (f"d{i}V", V), (f"d{i}OTf", OTf), (f"d{i}XhT", XhT)):
            dd = nc.declare_dram_parameter(nm, list(t[:].shape), fp32,
                                           isOutput=True)
            nc.gpsimd.dma_start(out=dd[:], in_=t[:])
    return XhT, XTnext


def _pma_q(nc, pools, w, extras):
    """PMA seed query (depends only on S + mab2 weights) - emitted early."""
    sbuf, psP = pools["sbuf"], pools["psP"]
    Wq, bq = w["w2q"], w["b2q"]
    ST, hmask = extras["st"], extras["hmask"]
    psq = psP.tile([128, QCH], fp32, tag="proj")
    nc.tensor.matmul(psq[:, 0:1], lhsT=Wq[:], rhs=ST[:, 0:1], start=True,
                     stop=True)
    QpTf = sbuf.tile([128, 1], fp32, tag="QpTf")
    QpTb = sbuf.tile([128, 1], bf16, tag="QpTb")
    nc.vector.tensor_tensor(out=QpTf[:], in0=psq[:, 0:1], in1=bq[:, 0:1],
                            op=ALU.add)
    nc.vector.tensor_copy(QpTb[:], QpTf[:])
    # Block-diagonal Qhat[d, h] = Qp^T[d] * (d//32 == h)
    Qhat = sbuf.tile([128, H], bf16, tag="Qhat")
    nc.vector.tensor_tensor(out=Qhat[:], in0=_bcast(QpTb[:, 0:1], H),
                            in1=hmask[:], op=ALU.mult)
    return QpTf, Qhat


class _PmaLocal:
    """PMA computed from this core's local 1024 keys; the pair's partial
    A@V vectors are AllReduced (tiny [128,1] fp32) before fc_o."""

    def __init__(self, nc, pools, w, extras, QpTf, Qhat):
        self.nc, self.pools = nc, pools
        self.w, self.extras = w, extras
        self.QpTf, self.Qhat = QpTf, Qhat
        sbuf = pools["sbuf"]
        self.KT = sbuf.tile([128, NQ], bf16, tag="pKT")
        self.V = sbuf.tile([128, NQ], bf16, tag="pV")
        self.Ap = sbuf.tile([128, 32], bf16, tag="Ap")

    def wave(self, XhT, wv):
        nc, pools = self.nc, self.pools
        psP, psF = pools["psP"], pools["psF"]
        Wk, Wv = self.w["w2k"], self.w["w2v"]
        bk, bvb = self.w["b2k"], self.w["b2v"]
        cs = wv * QCH
        ps = psP.tile([128, QCH], fp32, tag="proj")
        nc.tensor.matmul(ps[:], lhsT=Wk[:], rhs=XhT[:, cs:cs + QCH],
                         start=True, stop=True)
        nc.vector.tensor_tensor(out=self.KT[:, cs:cs + QCH], in0=ps[:],
                                in1=_bcast(bk[:, 0:1], QCH), op=ALU.add)
        for tl in range(4):
            t = 4 * wv + tl
            ps = psP.tile([128, QCH], fp32, tag="proj")
            nc.tensor.matmul(ps[:, 0:128], lhsT=XhT[:, t * 128:(t + 1) * 128],
                             rhs=Wv[:], start=True, stop=True)
            nc.vector.tensor_tensor(out=self.V[:, t * 128:(t + 1) * 128],
                                    in0=ps[:, 0:128], in1=bvb[:], op=ALU.add)
        Lp = psF.tile([128, QCH], fp32, tag="F")
        for tl in range(4):
            t = 4 * wv + tl
            nc.tensor.matmul(Lp[:, 4 * tl:4 * tl + 4],
                             lhsT=self.KT[:, t * 128:(t + 1) * 128],
                             rhs=self.Qhat[:], start=True, stop=True)
        nc.scalar.activation(self.Ap[:, 16 * wv:16 * wv + 16], Lp[:, 0:16],
                             SIG, scale=float(SCALE))

    def finish(self):
        nc, pools = self.nc, self.pools
        sbuf, psO, psF, dram = (pools["sbuf"], pools["psO"], pools["psF"],
                                pools["dram"])
        Wo, bo = self.w["w2o"], self.w["b2o"]
        pW, pb = self.extras["pw"], self.extras["pb"]
        # partial O^T over local keys, col-banded per head
        OpT_t = psO.tile([128, QCH], fp32, tag="OT")
        OpTps = OpT_t[:, 0:1]
        for h in range(H):
            for t in range(8):
                nc.tensor.matmul(
                    OpTps[32 * h:32 * h + 32, 0:1],
                    lhsT=self.V[:, 128 * t + 32 * h:128 * t + 32 * h + 32],
                    rhs=self.Ap[:, 4 * t + h:4 * t + h + 1],
                    start=(t == 0), stop=(t == 7),
                    tile_position=(0, 32 * h),
                    skip_group_check=True)
        AVp = sbuf.tile([128, 1], fp32, tag="AVp")
        nc.vector.tensor_copy(AVp[:], OpTps[:])
        cc_in = dram.tile([128, 1], fp32, tag="pcci")
        nc.sync.dma_start(out=cc_in[:], in_=AVp[:])
        cc_out = dram.tile([128, 1], fp32, tag="pcco")
        nc.gpsimd.collective_compute(
            "AllReduce", ALU.add, replica_groups=PAIRS,
            ins=[cc_in[:]], outs=[cc_out[:]])
        AVf = sbuf.tile([128, 1], fp32, tag="AVf")
        nc.sync.dma_start(out=AVf[:], in_=cc_out[:])
        OpTf = sbuf.tile([128, 1], fp32, tag="OpTf")
        OpTb = sbuf.tile([128, 1], bf16, tag="OpTb")
        nc.vector.tensor_tensor(out=OpTf[:], in0=AVf[:], in1=self.QpTf[:],
                                op=ALU.add)
        nc.vector.tensor_tensor(out=OpTb[:], in0=AVf[:], in1=self.QpTf[:],
                                op=ALU.add)
        FC2_t = psF.tile([128, QCH], fp32, tag="F")
        FC2 = FC2_t[:, 0:1]
        nc.tensor.matmul(FC2[:], lhsT=Wo[:], rhs=OpTb[:], start=True, stop=True)
        R2 = sbuf.tile([128, 1], fp32, tag="R2")
        nc.vector.tensor_scalar(out=R2[:], in0=FC2[:], scalar1=bo[:, 0:1],
                                scalar2=0.0, op0=ALU.add, op1=ALU.max)
        XpTb = sbuf.tile([128, 1], bf16, tag="XpTb")
        nc.vector.tensor_tensor(out=XpTb[:], in0=OpTf[:], in1=R2[:], op=ALU.add)
        OUT_t = psF.tile([128, QCH], fp32, tag="F")
        OUTps = OUT_t[0:1, 0:DOUT]
        nc.tensor.matmul(OUTps[:], lhsT=XpTb[:], rhs=pW[:], start=True,
                         stop=True)
        out_sb = sbuf.tile([1, DOUT], fp32, tag="out_sb")
        nc.vector.tensor_tensor(out=out_sb[:], in0=OUTps[:], in1=pb[:],
                                op=ALU.add)
        return out_sb


def build_program():
    nc = bass.Bass(num_devices=8)
    xt = nc.declare_dram_parameter("xt", [128, N], bf16, isOutput=False)
    xtq = nc.declare_dram_parameter("xtq", [128, NQ], bf16, isOutput=False)
    out_d = nc.declare_dram_parameter("out", [1, DOUT], fp32, isOutput=True)

    wshapes = {}
    for i in range(3):
        for k in ("q", "k", "v", "o"):
            wshapes[f"w{i}{k}"] = ([128, 128], bf16)
        wshapes[f"b{i}q"] = ([128, 1], fp32)
        wshapes[f"b{i}k"] = ([128, 1], fp32)
        wshapes[f"b{i}v"] = ([128, 128], fp32)  # pre-broadcast across partitions
        wshapes[f"b{i}o"] = ([128, 1], fp32)
    eshapes = {
        "st": ([128, 1], bf16),
        "hmask": ([128, H], bf16),
        "pw": ([128, DOUT], bf16),
        "pb": ([1, DOUT], fp32),
    }

    with tile.TileContext(nc) as tc:
        with (
            tc.tile_pool(name="sbuf", bufs=1) as sbuf,
            tc.tile_pool(name="sbufA", bufs=3) as sbufA,
            tc.tile_pool(name="psL", bufs=2, space="PSUM") as psL,
            tc.tile_pool(name="psO", bufs=1, space="PSUM") as psO,
            tc.tile_pool(name="psP", bufs=2, space="PSUM") as psP,
            tc.tile_pool(name="psF", bufs=1, space="PSUM") as psF,
            tc.tile_pool(name="dram", bufs=1, space="DRAM") as dram,
        ):
            pools = {"sbuf": sbuf, "sbufA": sbufA, "psL": psL, "psO": psO,
                     "psP": psP, "psF": psF, "dram": dram}

            # inputs: xt/xtq chunks on the HW-DGE queue, weights on SW-DGE
            XT0 = sbuf.tile([128, N], bf16, tag="XT0")
            XTq0 = sbuf.tile([128, NQ], bf16, tag="XTq0")
            nc.sync.dma_start(out=XTq0[:, 0:QCH], in_=xtq[:, 0:QCH])
            nc.sync.dma_start(out=XT0[:, 0:QCH], in_=xt[:, 0:QCH])
            nc.sync.dma_start(out=XTq0[:, QCH:NQ], in_=xtq[:, QCH:NQ])
            for c in range(1, 4):
                nc.sync.dma_start(out=XT0[:, c * QCH:(c + 1) * QCH],
                                  in_=xt[:, c * QCH:(c + 1) * QCH])
            w0 = {k: v for k, v in wshapes.items() if "0" in k}
            wrest = {k: v for k, v in wshapes.items() if "0" not in k}
            w = _load_weights(nc, sbuf, w0)
            w.update(_load_weights(nc, sbuf, wrest))
            extras = _load_weights(nc, sbuf, eshapes)
            # warm the ACT sigmoid table off the critical path
            warm = sbuf.tile([1, 1], fp32, tag="warm")
            nc.scalar.activation(warm[:], extras["pb"][0:1, 0:1], SIG)

            WAVES0 = [([0, 1], [0, 1, 2, 3, 4, 5, 6, 7]),
                      ([2, 3], [8, 9, 10, 11, 12, 13, 14, 15])]
            WAVES_AG = [([0, 2], [0, 1, 2, 3, 8, 9, 10, 11]),
                        ([1, 3], [4, 5, 6, 7, 12, 13, 14, 15])]
            Xh0, XT1 = _sab(nc, pools, XT0, XTq0, w, 0, "s0", WAVES0,
                            emit_ag=True)
            QpTf, Qhat = _pma_q(nc, pools, w, extras)
            pma = _PmaLocal(nc, pools, w, extras, QpTf, Qhat)
            post = {0: lambda XhT, qc: pma.wave(XhT, 0),
                    1: lambda XhT, qc: pma.wave(XhT, 1)}
            Xh1, _ = _sab(nc, pools, XT1, Xh0, w, 1, "s1", WAVES_AG,
                          emit_ag=False, post_qc=post)
            out_sb = pma.finish()
            nc.sync.dma_start(out=out_d[:], in_=out_sb[:])

    _fix_excess_waits(nc)
    return nc


_CACHE = {}


def _inputs_for_core(inputs, c):
    b, hf = c // 2, c % 2
    X = np.asarray(inputs["X"], dtype=np.float32)
    XT = np.ascontiguousarray(X[b].T).astype(ml_dtypes.bfloat16)
    m = {
        "xt": XT,
        "xtq": np.ascontiguousarray(XT[:, hf * NQ:(hf + 1) * NQ]),
        "st": np.ascontiguousarray(
            np.asarray(inputs["S"], np.float32).reshape(D, 1)
        ).astype(ml_dtypes.bfloat16),
        "hmask": (np.arange(128)[:, None] // 32 == np.arange(H)[None, :]
                  ).astype(ml_dtypes.bfloat16),
        "pw": np.ascontiguousarray(
            np.asarray(inputs["pW"], np.float32)).astype(ml_dtypes.bfloat16),
        "pb": np.asarray(inputs["pb"], np.float32).reshape(1, DOUT),
    }
    for i in range(3):
        for k in ("q", "k", "v", "o"):
            m[f"w{i}{k}"] = np.ascontiguousarray(
                np.asarray(inputs[f"m{i}_W{k}"], np.float32)
            ).astype(ml_dtypes.bfloat16)
        m[f"b{i}q"] = np.asarray(inputs[f"m{i}_bq"], np.float32).reshape(128, 1)
        m[f"b{i}k"] = np.asarray(inputs[f"m{i}_bk"], np.float32).reshape(128, 1)
        m[f"b{i}v"] = np.tile(
            np.asarray(inputs[f"m{i}_bv"], np.float32)[None, :], (128, 1))
        m[f"b{i}o"] = np.asarray(inputs[f"m{i}_bo"], np.float32).reshape(128, 1)
    return m


def kernel(**inputs) -> np.ndarray:
    if "nc" not in _CACHE:
        _CACHE["nc"] = build_program()
    nc = _CACHE["nc"]
    in_maps = [_inputs_for_core(inputs, c) for c in range(8)]
    res = run_bass_kernel_spmd(nc, in_maps, list(range(8)))
    out = np.stack([res.results[2 * b]["out"] for b in range(B)], axis=0)
    return out.astype(np.float32)  # [B, 1, DOUT]


# revision 17
# speedup vs baseline: 1.0150x; 1.0150x over previous
"""Set-Transformer encoder (2x SAB sigmoid-attention + PMA) on 8 TRN2 cores.

Sharding: core c handles batch b=c//2, query-half hf=c%2 (1024 of 2048 rows).
All data flows feature-major ([D=128 partitions, tokens]); the host supplies
X pre-transposed and pre-cast to bf16.  Between SAB layers each core pair
AllGathers its half of the layer output in two query-chunks, launched as
soon as each chunk is ready so the exchange hides under the remaining
attention work; the next layer processes the keys covered by the first
chunk before the second arrives (attention is permutation-invariant over
keys).  The PMA + final projection are computed redundantly by both cores
of a pair.

Matmul operands are bf16 (1 cycle/row on PE); accumulation and the residual
spine stay fp32.  The per-head (dh=32) QK matmuls use 32x32 tile_position
packing (8 concurrent tiles per 128-key group); AV contracts the full 128
keys with col-banded (M=32) matmuls accumulating O^T in place.
"""
import numpy as np
import ml_dtypes

import concourse.bass as bass
import concourse.tile as tile
from concourse import mybir
from concourse.bass_utils import run_bass_kernel_spmd

B, N, D, H, DH, DOUT = 4, 2048, 128, 4, 32, 256
NQ = N // 2          # queries per core
QCH = 512            # query chunk (matmul moving-dim)
NKT = N // 128       # 16 key tiles
SCALE = 1.0 / np.sqrt(np.float32(D))  # 1/sqrt(128) logit scale

fp32 = mybir.dt.float32
bf16 = mybir.dt.bfloat16
ALU = mybir.AluOpType
SIG = mybir.ActivationFunctionType.Sigmoid
PAIRS = [[0, 1], [2, 3], [4, 5], [6, 7]]
DEBUG_TAPS = False

# key-tile processing order when keys arrive via 2-chunk AllGather:
# AG chunk a carries each core's queries [0:512) -> global keys
# [0:512) u [1024:1536) = key tiles 0-3 and 8-11.
KT_ORDER_AG = [0, 1, 2, 3, 8, 9, 10, 11, 4, 5, 6, 7, 12, 13, 14, 15]
KCH_ORDER_AG = [0, 2, 1, 3]          # 512-col projection chunk order


def _fix_excess_waits(nc):
    """walrus accepts very few sync waits per instruction; hoist excess
    waits onto preceding same-engine NOPs (same stream => same semantics)."""
    for f in nc.m.functions:
        for bb in f.blocks:
            new_list = []
            for ins in bb.instructions:
                si = ins.sync_info
                cap = 2 if isinstance(ins, mybir.InstEventSemaphore) else 1
                if si is not None and len(si.on_wait) > cap:
                    waits = list(si.on_wait)
                    excess, kept = waits[:-cap], waits[-cap:]
                    for j, w in enumerate(excess):
                        nop = mybir.InstNoOp(
                            name=f"{ins.name}-presync{j}", ins=[], outs=[]
                        )
                        nop.engine = ins.engine
                        nop.sync_info = mybir.SyncInfo(on_wait=[w], on_update=[])
                        nc.register_instruction(nop)
                        new_list.append(nop)
                    ins.sync_info = mybir.SyncInfo(
                        on_wait=kept, on_update=list(si.on_update)
                    )
                new_list.append(ins)
            bb.instructions = new_list


def _bcast(ap, n):
    return ap.to_broadcast([ap.shape[0], n])


def _load_weights(nc, sbuf, shapes):
    tiles = {}
    for key, (shape, dt) in shapes.items():
        p = nc.declare_dram_parameter(key, shape, dt if dt == bf16 else fp32,
                                      isOutput=False)
        t = sbuf.tile(shape, dt, tag=f"in_{key}")
        nc.gpsimd.dma_start(out=t[:], in_=p[:])
        tiles[key] = t
    return tiles


def _proj_q(nc, pools, XTq, w, i, tagp):
    sbuf, psP = pools["sbuf"], pools["psP"]
    Wq, bq = w[f"w{i}q"], w[f"b{i}q"]
    QTf = sbuf.tile([128, NQ], fp32, tag=f"{tagp}QTf")
    QTb = sbuf.tile([128, NQ], bf16, tag=f"{tagp}QTb")
    for c in range(2):
        ps = psP.tile([128, QCH], fp32, tag="proj")
        nc.tensor.matmul(ps[:], lhsT=Wq[:], rhs=XTq[:, c * QCH:(c + 1) * QCH],
                         start=True, stop=True)
        nc.vector.tensor_tensor(
            out=QTf[:, c * QCH:(c + 1) * QCH], in0=ps[:],
            in1=_bcast(bq[:, 0:1], QCH), op=ALU.add)
        nc.vector.tensor_tensor(
            out=QTb[:, c * QCH:(c + 1) * QCH], in0=ps[:],
            in1=_bcast(bq[:, 0:1], QCH), op=ALU.add)
    return QTf, QTb


def _proj_kv_wave(nc, pools, XTfull, w, i, KT, V, kchs, kts):
    """K^T chunks + V tiles for one wave of arrived keys."""
    psP = pools["psP"]
    Wk, Wv = w[f"w{i}k"], w[f"w{i}v"]
    bk, bvb = w[f"b{i}k"], w[f"b{i}v"]
    for c in kchs:
        ps = psP.tile([128, QCH], fp32, tag="proj")
        nc.tensor.matmul(ps[:], lhsT=Wk[:], rhs=XTfull[:, c * QCH:(c + 1) * QCH],
                         start=True, stop=True)
        nc.vector.tensor_tensor(
            out=KT[:, c * QCH:(c + 1) * QCH], in0=ps[:],
            in1=_bcast(bk[:, 0:1], QCH), op=ALU.add)
    for t in kts:
        ps = psP.tile([128, QCH], fp32, tag="proj")
        nc.tensor.matmul(ps[:, 0:128], lhsT=XTfull[:, t * 128:(t + 1) * 128],
                         rhs=Wv[:], start=True, stop=True)
        nc.vector.tensor_tensor(out=V[:, t * 128:(t + 1) * 128], in0=ps[:, 0:128],
                                in1=bvb[:], op=ALU.add)


def _sab(nc, pools, XTfull, XTq, w, i, tagp, waves, emit_ag, post_qc=None):
    """One SAB layer; returns (XhT_half, XTnext or None).

    waves: list of (kch_list, kt_list) -- keys grouped by arrival order.
    post_qc: optional {qc: fn(XhT, qc)} called after each query chunk."""
    sbuf, sbufA, psL, psO, psF, dram = (
        pools["sbuf"], pools["sbufA"], pools["psL"], pools["psO"], pools["psF"],
        pools["dram"],
    )
    Wo, bo = w[f"w{i}o"], w[f"b{i}o"]
    kt_order = [t for _, kts in waves for t in kts]
    QTf, QTb = _proj_q(nc, pools, XTq, w, i, tagp)
    KT = sbuf.tile([128, N], bf16, tag=f"{tagp}KT")
    V = sbuf.tile([128, N], bf16, tag=f"{tagp}V")

    XhT = sbuf.tile([128, NQ], bf16, tag=f"{tagp}XhT")
    OTf = sbuf.tile([128, NQ], fp32, tag=f"{tagp}OTf")
    OTb = sbuf.tile([128, NQ], bf16, tag=f"{tagp}OTb")
    XTnext = None
    if emit_ag:
        XTnext = sbuf.tile([128, N], bf16, tag=f"{tagp}XTn")

    for qc in range(2):
        qs = qc * QCH
        OTps = psO.tile([128, QCH], fp32, tag="OT")
        groups = [(kt, hp) for kt in kt_order for hp in range(2)]
        Ltiles = {}
        # wave w's projections are emitted just before its first group
        # (first qc pass only); the scheduler starts them as keys arrive.
        proj_at = {}
        if qc == 0:
            gidx = 0
            for kchs, kts in waves:
                proj_at[gidx] = (kchs, kts)
                gidx += 2 * len(kts)

        def emit_qk(g):
            kt, hp = groups[g]
            L = psL.tile([128, 1024], fp32, tag="L")
            Ltiles[g] = L
            for h in (2 * hp, 2 * hp + 1):
                for j in range(4):
                    nc.tensor.matmul(
                        out=L[32 * j:32 * j + 32,
                              QCH * (h - 2 * hp):QCH * (h - 2 * hp) + QCH],
                        lhsT=KT[32 * h:32 * h + 32,
                                128 * kt + 32 * j:128 * kt + 32 * j + 32],
                        rhs=QTb[32 * h:32 * h + 32, qs:qs + QCH],
                        start=True, stop=True,
                        tile_position=(32 * h, 32 * j))

        ng = len(groups)
        pending_v = []

        def maybe_wave(gi):
            if gi in proj_at:
                kchs, kts = proj_at[gi]
                _proj_kv_wave(nc, pools, XTfull, w, i, KT, V, kchs, [])
                pending_v.append(kts)

        maybe_wave(0)
        emit_qk(0)
        if pending_v:
            _proj_kv_wave(nc, pools, XTfull, w, i, KT, V, [], pending_v.pop())
        for g in range(ng):
            kt, hp = groups[g]
            maybe_wave(g + 1)
            if g + 1 < ng:
                emit_qk(g + 1)
            if pending_v:
                _proj_kv_wave(nc, pools, XTfull, w, i, KT, V, [],
                              pending_v.pop())
            A = sbufA.tile([128, 1024], bf16, tag="A")
            nc.scalar.activation(A[:], Ltiles.pop(g)[:], SIG, scale=float(SCALE))
            # AV: A holds the full 128 keys of tile kt on partitions;
            # contract K=128 with one col-banded matmul per head.
            for h in (2 * hp, 2 * hp + 1):
                nc.tensor.matmul(
                    out=OTps[32 * h:32 * h + 32, 0:QCH],
                    lhsT=V[:, 128 * kt + 32 * h:128 * kt + 32 * h + 32],
                    rhs=A[:, QCH * (h - 2 * hp):QCH * (h - 2 * hp) + QCH],
                    start=(g // 2 == 0), stop=(g // 2 == NKT - 1),
                    tile_position=(0, 32 * h),
                    skip_group_check=True)

        # O = Qp + A@V ; Xh = O + relu(O @ Wo + bo)
        nc.vector.tensor_tensor(out=OTf[:, qs:qs + QCH], in0=OTps[:],
                                in1=QTf[:, qs:qs + QCH], op=ALU.add)
        nc.vector.tensor_tensor(out=OTb[:, qs:qs + QCH], in0=OTps[:],
                                in1=QTf[:, qs:qs + QCH], op=ALU.add)
        FC = psF.tile([128, QCH], fp32, tag="F")
        nc.tensor.matmul(FC[:], lhsT=Wo[:], rhs=OTb[:, qs:qs + QCH],
                         start=True, stop=True)
        R = sbuf.tile([128, QCH], fp32, tag="R")
        nc.vector.tensor_scalar(out=R[:], in0=FC[:], scalar1=bo[:, 0:1],
                                scalar2=0.0, op0=ALU.add, op1=ALU.max)
        nc.vector.tensor_tensor(out=XhT[:, qs:qs + QCH], in0=OTf[:, qs:qs + QCH],
                                in1=R[:], op=ALU.add)

        if emit_ag:
            # exchange this query chunk with the pair core right away
            cc_in = dram.tile([128, QCH], bf16, tag=f"{tagp}cci{qc}")
            nc.sync.dma_start(out=cc_in[:], in_=XhT[:, qs:qs + QCH])
            cc_out = dram.tile([256, QCH], bf16, tag=f"{tagp}cco{qc}")
            nc.gpsimd.collective_compute(
                "AllGather", ALU.bypass, replica_groups=PAIRS,
                ins=[cc_in[:]], outs=[cc_out[:]])
            # global columns: rank0 rows -> [qs:qs+512), rank1 -> [1024+qs:...)
            nc.sync.dma_start(out=XTnext[:, qs:qs + QCH], in_=cc_out[0:128, :])
            nc.sync.dma_start(out=XTnext[:, NQ + qs:NQ + qs + QCH],
                              in_=cc_out[128:256, :])
        if post_qc and qc in post_qc:
            post_qc[qc](XhT, qc)

    if DEBUG_TAPS:
        for nm, t in ((f"d{i}KT", KT), (f"d{i}QTb", QTb), (f"d{i}QTf", QTf),
                      (f"d{i}V", V), (f"d{i}OTf", OTf), (f"d{i}XhT", XhT)):
            dd = nc.declare_dram_parameter(nm, list(t[:].shape), fp32,
                                           isOutput=True)
            nc.gpsimd.dma_start(out=dd[:], in_=t[:])
    return XhT, XTnext


def _pma_q(nc, pools, w, extras):
    """PMA seed query (depends only on S + mab2 weights) - emitted early."""
    sbuf, psP = pools["sbuf"], pools["psP"]
    Wq, bq = w["w2q"], w["b2q"]
    ST, hmask = extras["st"], extras["hmask"]
    psq = psP.tile([128, QCH], fp32, tag="proj")
    nc.tensor.matmul(psq[:, 0:1], lhsT=Wq[:], rhs=ST[:, 0:1], start=True,
                     stop=True)
    QpTf = sbuf.tile([128, 1], fp32, tag="QpTf")
    QpTb = sbuf.tile([128, 1], bf16, tag="QpTb")
    nc.vector.tensor_tensor(out=QpTf[:], in0=psq[:, 0:1], in1=bq[:, 0:1],
                            op=ALU.add)
    nc.vector.tensor_copy(QpTb[:], QpTf[:])
    # Block-diagonal Qhat[d, h] = Qp^T[d] * (d//32 == h)
    Qhat = sbuf.tile([128, H], bf16, tag="Qhat")
    nc.vector.tensor_tensor(out=Qhat[:], in0=_bcast(QpTb[:, 0:1], H),
                            in1=hmask[:], op=ALU.mult)
    return QpTf, Qhat


class _PmaLocal:
    """PMA computed from this core's local 1024 keys; the pair's partial
    A@V vectors are AllReduced (tiny [128,1] fp32) before fc_o."""

    def __init__(self, nc, pools, w, extras, QpTf, Qhat):
        self.nc, self.pools = nc, pools
        self.w, self.extras = w, extras
        self.QpTf, self.Qhat = QpTf, Qhat
        sbuf = pools["sbuf"]
        self.KT = sbuf.tile([128, NQ], bf16, tag="pKT")
        self.V = sbuf.tile([128, NQ], bf16, tag="pV")
        self.Ap = sbuf.tile([128, 32], bf16, tag="Ap")

    def wave(self, XhT, wv):
        nc, pools = self.nc, self.pools
        psP, psF = pools["psP"], pools["psF"]
        Wk, Wv = self.w["w2k"], self.w["w2v"]
        bk, bvb = self.w["b2k"], self.w["b2v"]
        cs = wv * QCH
        ps = psP.tile([128, QCH], fp32, tag="proj")
        nc.tensor.matmul(ps[:], lhsT=Wk[:], rhs=XhT[:, cs:cs + QCH],
                         start=True, stop=True)
        nc.vector.tensor_tensor(out=self.KT[:, cs:cs + QCH], in0=ps[:],
                                in1=_bcast(bk[:, 0:1], QCH), op=ALU.add)
        for tl in range(4):
            t = 4 * wv + tl
            ps = psP.tile([128, QCH], fp32, tag="proj")
            nc.tensor.matmul(ps[:, 0:128], lhsT=XhT[:, t * 128:(t + 1) * 128],
                             rhs=Wv[:], start=True, stop=True)
            nc.vector.tensor_tensor(out=self.V[:, t * 128:(t + 1) * 128],
                                    in0=ps[:, 0:128], in1=bvb[:], op=ALU.add)
        Lp = psF.tile([128, QCH], fp32, tag="F")
        for tl in range(4):
            t = 4 * wv + tl
            nc.tensor.matmul(Lp[:, 4 * tl:4 * tl + 4],
                             lhsT=self.KT[:, t * 128:(t + 1) * 128],
                             rhs=self.Qhat[:], start=True, stop=True)
        nc.scalar.activation(self.Ap[:, 16 * wv:16 * wv + 16], Lp[:, 0:16],
                             SIG, scale=float(SCALE))
        OpT_t = self.pools["psO"].tile([128, QCH], fp32, tag="OT")
        OpTps = OpT_t[:, 0:1]
        for h in range(H):
            for tl in range(4):
                t = 4 * wv + tl
                nc.tensor.matmul(
                    OpTps[32 * h:32 * h + 32, 0:1],
                    lhsT=self.V[:, 128 * t + 32 * h:128 * t + 32 * h + 32],
                    rhs=self.Ap[:, 4 * t + h:4 * t + h + 1],
                    start=(tl == 0), stop=(tl == 3),
                    tile_position=(0, 32 * h),
                    skip_group_check=True)
        # drain the per-wave partial so the PSUM bank frees immediately
        if wv == 0:
            self.AVa = self.pools["sbuf"].tile([128, 1], fp32, tag="AVa")
            nc.vector.tensor_copy(self.AVa[:], OpTps[:])
        else:
            self.AVb = self.pools["sbuf"].tile([128, 1], fp32, tag="AVb")
            nc.vector.tensor_tensor(out=self.AVb[:], in0=OpTps[:],
                                    in1=self.AVa[:], op=ALU.add)

    def finish(self):
        nc, pools = self.nc, self.pools
        sbuf, psO, psF, dram = (pools["sbuf"], pools["psO"], pools["psF"],
                                pools["dram"])
        Wo, bo = self.w["w2o"], self.w["b2o"]
        pW, pb = self.extras["pw"], self.extras["pb"]
        cc_in = dram.tile([128, 1], fp32, tag="pcci")
        nc.sync.dma_start(out=cc_in[:], in_=self.AVb[:])
        cc_out = dram.tile([128, 1], fp32, tag="pcco")
        nc.gpsimd.collective_compute(
            "AllReduce", ALU.add, replica_groups=PAIRS,
            ins=[cc_in[:]], outs=[cc_out[:]])
        AVf = sbuf.tile([128, 1], fp32, tag="AVf")
        nc.sync.dma_start(out=AVf[:], in_=cc_out[:])
        OpTf = sbuf.tile([128, 1], fp32, tag="OpTf")
        OpTb = sbuf.tile([128, 1], bf16, tag="OpTb")
        nc.vector.tensor_tensor(out=OpTf[:], in0=AVf[:], in1=self.QpTf[:],
                                op=ALU.add)
        nc.vector.tensor_tensor(out=OpTb[:], in0=AVf[:], in1=self.QpTf[:],
                                op=ALU.add)
        FC2_t = psF.tile([128, QCH], fp32, tag="F")
        FC2 = FC2_t[:, 0:1]
        nc.tensor.matmul(FC2[:], lhsT=Wo[:], rhs=OpTb[:], start=True, stop=True)
        R2 = sbuf.tile([128, 1], fp32, tag="R2")
        nc.scalar.activation(R2[:], FC2[:], mybir.ActivationFunctionType.Relu,
                             bias=bo[:, 0:1])
        XpTb = sbuf.tile([128, 1], bf16, tag="XpTb")
        nc.vector.tensor_tensor(out=XpTb[:], in0=OpTf[:], in1=R2[:], op=ALU.add)
        OUT_t = psF.tile([128, QCH], fp32, tag="F")
        OUTps = OUT_t[0:1, 0:DOUT]
        nc.tensor.matmul(OUTps[:], lhsT=XpTb[:], rhs=pW[:], start=True,
                         stop=True)
        out_sb = sbuf.tile([1, DOUT], fp32, tag="out_sb")
        nc.vector.tensor_tensor(out=out_sb[:], in0=OUTps[:], in1=pb[:],
                                op=ALU.add)
        return out_sb


def build_program():
    nc = bass.Bass(num_devices=8)
    xt = nc.declare_dram_parameter("xt", [128, N], bf16, isOutput=False)
    xtq = nc.declare_dram_parameter("xtq", [128, NQ], bf16, isOutput=False)
    out_d = nc.declare_dram_parameter("out", [1, DOUT], fp32, isOutput=True)

    wshapes = {}
    for i in range(3):
        for k in ("q", "k", "v", "o"):
            wshapes[f"w{i}{k}"] = ([128, 128], bf16)
        wshapes[f"b{i}q"] = ([128, 1], fp32)
        wshapes[f"b{i}k"] = ([128, 1], fp32)
        wshapes[f"b{i}v"] = ([128, 128], fp32)  # pre-broadcast across partitions
        wshapes[f"b{i}o"] = ([128, 1], fp32)
    eshapes = {
        "st": ([128, 1], bf16),
        "hmask": ([128, H], bf16),
        "pw": ([128, DOUT], bf16),
        "pb": ([1, DOUT], fp32),
    }

    with tile.TileContext(nc) as tc:
        with (
            tc.tile_pool(name="sbuf", bufs=1) as sbuf,
            tc.tile_pool(name="sbufA", bufs=3) as sbufA,
            tc.tile_pool(name="psL", bufs=2, space="PSUM") as psL,
            tc.tile_pool(name="psO", bufs=1, space="PSUM") as psO,
            tc.tile_pool(name="psP", bufs=2, space="PSUM") as psP,
            tc.tile_pool(name="psF", bufs=1, space="PSUM") as psF,
            tc.tile_pool(name="dram", bufs=1, space="DRAM") as dram,
        ):
            pools = {"sbuf": sbuf, "sbufA": sbufA, "psL": psL, "psO": psO,
                     "psP": psP, "psF": psF, "dram": dram}

            # inputs: xt/xtq chunks on the HW-DGE queue, weights on SW-DGE
            XT0 = sbuf.tile([128, N], bf16, tag="XT0")
            XTq0 = sbuf.tile([128, NQ], bf16, tag="XTq0")
            nc.sync.dma_start(out=XTq0[:, 0:QCH], in_=xtq[:, 0:QCH])
            nc.sync.dma_start(out=XT0[:, 0:QCH], in_=xt[:, 0:QCH])
            nc.sync.dma_start(out=XTq0[:, QCH:NQ], in_=xtq[:, QCH:NQ])
            for c in range(1, 4):
                nc.sync.dma_start(out=XT0[:, c * QCH:(c + 1) * QCH],
                                  in_=xt[:, c * QCH:(c + 1) * QCH])
            w0 = {k: v for k, v in wshapes.items() if "0" in k}
            wrest = {k: v for k, v in wshapes.items() if "0" not in k}
            w = _load_weights(nc, sbuf, w0)
            w.update(_load_weights(nc, sbuf, wrest))
            extras = _load_weights(nc, sbuf, eshapes)
            # warm the ACT sigmoid table off the critical path
            warm = sbuf.tile([1, 1], fp32, tag="warm")
            nc.scalar.activation(warm[:], extras["pb"][0:1, 0:1], SIG)

            WAVES0 = [([0, 1], [0, 1, 2, 3, 4, 5, 6, 7]),
                      ([2, 3], [8, 9, 10, 11, 12, 13, 14, 15])]
            WAVES_AG = [([0, 2], [0, 1, 2, 3, 8, 9, 10, 11]),
                        ([1, 3], [4, 5, 6, 7, 12, 13, 14, 15])]
            Xh0, XT1 = _sab(nc, pools, XT0, XTq0, w, 0, "s0", WAVES0,
                            emit_ag=True)
            QpTf, Qhat = _pma_q(nc, pools, w, extras)
            pma = _PmaLocal(nc, pools, w, extras, QpTf, Qhat)
            post = {0: lambda XhT, qc: pma.wave(XhT, 0),
                    1: lambda XhT, qc: pma.wave(XhT, 1)}
            Xh1, _ = _sab(nc, pools, XT1, Xh0, w, 1, "s1", WAVES_AG,
                          emit_ag=False, post_qc=post)
            out_sb = pma.finish()
            nc.sync.dma_start(out=out_d[:], in_=out_sb[:])

    _fix_excess_waits(nc)
    return nc


_CACHE = {}


def _inputs_for_core(inputs, c):
    b, hf = c // 2, c % 2
    X = np.asarray(inputs["X"], dtype=np.float32)
    XT = np.ascontiguousarray(X[b].T).astype(ml_dtypes.bfloat16)
    m = {
        "xt": XT,
        "xtq": np.ascontiguousarray(XT[:, hf * NQ:(hf + 1) * NQ]),
        "st": np.ascontiguousarray(
            np.asarray(inputs["S"], np.float32).reshape(D, 1)
        ).astype(ml_dtypes.bfloat16),
        "hmask": (np.arange(128)[:, None] // 32 == np.arange(H)[None, :]
                  ).astype(ml_dtypes.bfloat16),
        "pw": np.ascontiguousarray(
            np.asarray(inputs["pW"], np.float32)).astype(ml_dtypes.bfloat16),
        "pb": np.asarray(inputs["pb"], np.float32).reshape(1, DOUT),
    }
    for i in range(3):
        for k in ("q", "k", "v", "o"):
            m[f"w{i}{k}"] = np.ascontiguousarray(
                np.asarray(inputs[f"m{i}_W{k}"], np.float32)
            ).astype(ml_dtypes.bfloat16)
        m[f"b{i}q"] = np.asarray(inputs[f"m{i}_bq"], np.float32).reshape(128, 1)
        m[f"b{i}k"] = np.asarray(inputs[f"m{i}_bk"], np.float32).reshape(128, 1)
        m[f"b{i}v"] = np.tile(
            np.asarray(inputs[f"m{i}_bv"], np.float32)[None, :], (128, 1))
        m[f"b{i}o"] = np.asarray(inputs[f"m{i}_bo"], np.float32).reshape(128, 1)
    return m


def kernel(**inputs) -> np.ndarray:
    if "nc" not in _CACHE:
        _CACHE["nc"] = build_program()
    nc = _CACHE["nc"]
    in_maps = [_inputs_for_core(inputs, c) for c in range(8)]
    res = run_bass_kernel_spmd(nc, in_maps, list(range(8)))
    out = np.stack([res.results[2 * b]["out"] for b in range(B)], axis=0)
    return out.astype(np.float32)  # [B, 1, DOUT]


# revision 18
# speedup vs baseline: 1.0262x; 1.0111x over previous
"""Set-Transformer encoder (2x SAB sigmoid-attention + PMA) on 8 TRN2 cores.

Sharding: core c handles batch b=c//2, query-half hf=c%2 (1024 of 2048 rows).
All data flows feature-major ([D=128 partitions, tokens]); the host supplies
X pre-transposed and pre-cast to bf16.  Between SAB layers each core pair
AllGathers its half of the layer output in two query-chunks, launched as
soon as each chunk is ready so the exchange hides under the remaining
attention work; the next layer processes the keys covered by the first
chunk before the second arrives (attention is permutation-invariant over
keys).  The PMA + final projection are computed redundantly by both cores
of a pair.

Matmul operands are bf16 (1 cycle/row on PE); accumulation and the residual
spine stay fp32.  The per-head (dh=32) QK matmuls use 32x32 tile_position
packing (8 concurrent tiles per 128-key group); AV contracts the full 128
keys with col-banded (M=32) matmuls accumulating O^T in place.
"""
import numpy as np
import ml_dtypes

import concourse.bass as bass
import concourse.tile as tile
from concourse import mybir
from concourse.bass_utils import run_bass_kernel_spmd

B, N, D, H, DH, DOUT = 4, 2048, 128, 4, 32, 256
NQ = N // 2          # queries per core
QCH = 512            # query chunk (matmul moving-dim)
NKT = N // 128       # 16 key tiles
SCALE = 1.0 / np.sqrt(np.float32(D))  # 1/sqrt(128) logit scale

fp32 = mybir.dt.float32
bf16 = mybir.dt.bfloat16
ALU = mybir.AluOpType
SIG = mybir.ActivationFunctionType.Sigmoid
PAIRS = [[0, 1], [2, 3], [4, 5], [6, 7]]
DEBUG_TAPS = False

# key-tile processing order when keys arrive via 2-chunk AllGather:
# AG chunk a carries each core's queries [0:512) -> global keys
# [0:512) u [1024:1536) = key tiles 0-3 and 8-11.
KT_ORDER_AG = [0, 1, 2, 3, 8, 9, 10, 11, 4, 5, 6, 7, 12, 13, 14, 15]
KCH_ORDER_AG = [0, 2, 1, 3]          # 512-col projection chunk order


def _fix_excess_waits(nc):
    """walrus accepts very few sync waits per instruction; hoist excess
    waits onto preceding same-engine NOPs (same stream => same semantics)."""
    for f in nc.m.functions:
        for bb in f.blocks:
            new_list = []
            for ins in bb.instructions:
                si = ins.sync_info
                cap = 2 if isinstance(ins, mybir.InstEventSemaphore) else 1
                if si is not None and len(si.on_wait) > cap:
                    waits = list(si.on_wait)
                    excess, kept = waits[:-cap], waits[-cap:]
                    for j, w in enumerate(excess):
                        nop = mybir.InstNoOp(
                            name=f"{ins.name}-presync{j}", ins=[], outs=[]
                        )
                        nop.engine = ins.engine
                        nop.sync_info = mybir.SyncInfo(on_wait=[w], on_update=[])
                        nc.register_instruction(nop)
                        new_list.append(nop)
                    ins.sync_info = mybir.SyncInfo(
                        on_wait=kept, on_update=list(si.on_update)
                    )
                new_list.append(ins)
            bb.instructions = new_list


def _bcast(ap, n):
    return ap.to_broadcast([ap.shape[0], n])


def _load_weights(nc, sbuf, shapes):
    tiles = {}
    for key, (shape, dt) in shapes.items():
        p = nc.declare_dram_parameter(key, shape, dt if dt == bf16 else fp32,
                                      isOutput=False)
        t = sbuf.tile(shape, dt, tag=f"in_{key}")
        nc.gpsimd.dma_start(out=t[:], in_=p[:])
        tiles[key] = t
    return tiles


def _proj_q(nc, pools, XTq, w, i, tagp):
    sbuf, psP = pools["sbuf"], pools["psP"]
    Wq, bq = w[f"w{i}q"], w[f"b{i}q"]
    QTf = sbuf.tile([128, NQ], fp32, tag=f"{tagp}QTf")
    QTb = sbuf.tile([128, NQ], bf16, tag=f"{tagp}QTb")
    for c in range(2):
        ps = psP.tile([128, QCH], fp32, tag="proj")
        nc.tensor.matmul(ps[:], lhsT=Wq[:], rhs=XTq[:, c * QCH:(c + 1) * QCH],
                         start=True, stop=True)
        nc.vector.tensor_tensor(
            out=QTf[:, c * QCH:(c + 1) * QCH], in0=ps[:],
            in1=_bcast(bq[:, 0:1], QCH), op=ALU.add)
        nc.vector.tensor_tensor(
            out=QTb[:, c * QCH:(c + 1) * QCH], in0=ps[:],
            in1=_bcast(bq[:, 0:1], QCH), op=ALU.add)
    return QTf, QTb


def _proj_kv_wave(nc, pools, XTfull, w, i, KT, V, kchs, kts):
    """K^T chunks + V tiles for one wave of arrived keys."""
    psP = pools["psP"]
    Wk, Wv = w[f"w{i}k"], w[f"w{i}v"]
    bk, bvb = w[f"b{i}k"], w[f"b{i}v"]
    for c in kchs:
        ps = psP.tile([128, QCH], fp32, tag="proj")
        nc.tensor.matmul(ps[:], lhsT=Wk[:], rhs=XTfull[:, c * QCH:(c + 1) * QCH],
                         start=True, stop=True)
        nc.vector.tensor_tensor(
            out=KT[:, c * QCH:(c + 1) * QCH], in0=ps[:],
            in1=_bcast(bk[:, 0:1], QCH), op=ALU.add)
    for t in kts:
        ps = psP.tile([128, QCH], fp32, tag="proj")
        nc.tensor.matmul(ps[:, 0:128], lhsT=XTfull[:, t * 128:(t + 1) * 128],
                         rhs=Wv[:], start=True, stop=True)
        nc.vector.tensor_tensor(out=V[:, t * 128:(t + 1) * 128], in0=ps[:, 0:128],
                                in1=bvb[:], op=ALU.add)


def _sab(nc, pools, XTfull, XTq, w, i, tagp, waves, emit_ag, post_qc=None):
    """One SAB layer; returns (XhT_half, XTnext or None).

    waves: list of (kch_list, kt_list) -- keys grouped by arrival order.
    post_qc: optional {qc: fn(XhT, qc)} called after each query chunk."""
    sbuf, sbufA, psL, psO, psF, dram = (
        pools["sbuf"], pools["sbufA"], pools["psL"], pools["psO"], pools["psF"],
        pools["dram"],
    )
    Wo, bo = w[f"w{i}o"], w[f"b{i}o"]
    kt_order = [t for _, kts in waves for t in kts]
    QTf, QTb = _proj_q(nc, pools, XTq, w, i, tagp)
    KT = sbuf.tile([128, N], bf16, tag=f"{tagp}KT")
    V = sbuf.tile([128, N], bf16, tag=f"{tagp}V")

    XhT = sbuf.tile([128, NQ], bf16, tag=f"{tagp}XhT")
    OTf = sbuf.tile([128, NQ], fp32, tag=f"{tagp}OTf")
    OTb = sbuf.tile([128, NQ], bf16, tag=f"{tagp}OTb")
    XTnext = None
    if emit_ag:
        XTnext = sbuf.tile([128, N], bf16, tag=f"{tagp}XTn")

    for qc in range(2):
        qs = qc * QCH
        OTps = psO.tile([128, QCH], fp32, tag="OT")
        groups = [(kt, hp) for kt in kt_order for hp in range(2)]
        Ltiles = {}
        # wave w's projections are emitted just before its first group
        # (first qc pass only); the scheduler starts them as keys arrive.
        proj_at = {}
        if qc == 0:
            gidx = 0
            for kchs, kts in waves:
                proj_at[gidx] = (kchs, kts)
                gidx += 2 * len(kts)

        def emit_qk(g):
            kt, hp = groups[g]
            L = psL.tile([128, 1024], fp32, tag="L")
            Ltiles[g] = L
            for h in (2 * hp, 2 * hp + 1):
                for j in range(4):
                    nc.tensor.matmul(
                        out=L[32 * j:32 * j + 32,
                              QCH * (h - 2 * hp):QCH * (h - 2 * hp) + QCH],
                        lhsT=KT[32 * h:32 * h + 32,
                                128 * kt + 32 * j:128 * kt + 32 * j + 32],
                        rhs=QTb[32 * h:32 * h + 32, qs:qs + QCH],
                        start=True, stop=True,
                        tile_position=(32 * h, 32 * j))

        ng = len(groups)
        pending_v = []

        def maybe_wave(gi):
            if gi in proj_at:
                kchs, kts = proj_at[gi]
                _proj_kv_wave(nc, pools, XTfull, w, i, KT, V, kchs, [])
                pending_v.append(kts)

        maybe_wave(0)
        emit_qk(0)
        if pending_v:
            _proj_kv_wave(nc, pools, XTfull, w, i, KT, V, [], pending_v.pop())
        for g in range(ng):
            kt, hp = groups[g]
            maybe_wave(g + 1)
            if g + 1 < ng:
                emit_qk(g + 1)
            if pending_v:
                _proj_kv_wave(nc, pools, XTfull, w, i, KT, V, [],
                              pending_v.pop())
            A = sbufA.tile([128, 1024], bf16, tag="A")
            nc.scalar.activation(A[:], Ltiles.pop(g)[:], SIG, scale=float(SCALE))
            # AV: A holds the full 128 keys of tile kt on partitions;
            # contract K=128 with one col-banded matmul per head.
            for h in (2 * hp, 2 * hp + 1):
                nc.tensor.matmul(
                    out=OTps[32 * h:32 * h + 32, 0:QCH],
                    lhsT=V[:, 128 * kt + 32 * h:128 * kt + 32 * h + 32],
                    rhs=A[:, QCH * (h - 2 * hp):QCH * (h - 2 * hp) + QCH],
                    start=(g // 2 == 0), stop=(g // 2 == NKT - 1),
                    tile_position=(0, 32 * h),
                    skip_group_check=True)

        # O = Qp + A@V ; Xh = O + relu(O @ Wo + bo)
        nc.vector.tensor_tensor(out=OTf[:, qs:qs + QCH], in0=OTps[:],
                                in1=QTf[:, qs:qs + QCH], op=ALU.add)
        nc.vector.tensor_tensor(out=OTb[:, qs:qs + QCH], in0=OTps[:],
                                in1=QTf[:, qs:qs + QCH], op=ALU.add)
        FC = psF.tile([128, QCH], fp32, tag="F")
        nc.tensor.matmul(FC[:], lhsT=Wo[:], rhs=OTb[:, qs:qs + QCH],
                         start=True, stop=True)
        R = sbuf.tile([128, QCH], fp32, tag="R")
        nc.vector.tensor_scalar(out=R[:], in0=FC[:], scalar1=bo[:, 0:1],
                                scalar2=0.0, op0=ALU.add, op1=ALU.max)
        nc.vector.tensor_tensor(out=XhT[:, qs:qs + QCH], in0=OTf[:, qs:qs + QCH],
                                in1=R[:], op=ALU.add)

        if emit_ag:
            # exchange this query chunk with the pair core right away
            cc_in = dram.tile([128, QCH], bf16, tag=f"{tagp}cci{qc}")
            nc.sync.dma_start(out=cc_in[:], in_=XhT[:, qs:qs + QCH])
            cc_out = dram.tile([256, QCH], bf16, tag=f"{tagp}cco{qc}")
            nc.gpsimd.collective_compute(
                "AllGather", ALU.bypass, replica_groups=PAIRS,
                ins=[cc_in[:]], outs=[cc_out[:]])
            # global columns: rank0 rows -> [qs:qs+512), rank1 -> [1024+qs:...)
            nc.sync.dma_start(out=XTnext[:, qs:qs + QCH], in_=cc_out[0:128, :])
            nc.sync.dma_start(out=XTnext[:, NQ + qs:NQ + qs + QCH],
                              in_=cc_out[128:256, :])
        if post_qc and qc in post_qc:
            post_qc[qc](XhT, qc)

    if DEBUG_TAPS:
        for nm, t in ((f"d{i}KT", KT), (f"d{i}QTb", QTb), (f"d{i}QTf", QTf),
                      (f"d{i}V", V), (f"d{i}OTf", OTf), (f"d{i}XhT", XhT)):
            dd = nc.declare_dram_parameter(nm, list(t[:].shape), fp32,
                                           isOutput=True)
            nc.gpsimd.dma_start(out=dd[:], in_=t[:])
    return XhT, XTnext


def _pma_q(nc, pools, w, extras):
    """PMA seed query (depends only on S + mab2 weights) - emitted early."""
    sbuf, psP = pools["sbuf"], pools["psP"]
    Wq, bq = w["w2q"], w["b2q"]
    ST, hmask = extras["st"], extras["hmask"]
    psq = psP.tile([128, QCH], fp32, tag="proj")
    nc.tensor.matmul(psq[:, 0:1], lhsT=Wq[:], rhs=ST[:, 0:1], start=True,
                     stop=True)
    QpTf = sbuf.tile([128, 1], fp32, tag="QpTf")
    QpTb = sbuf.tile([128, 1], bf16, tag="QpTb")
    nc.vector.tensor_tensor(out=QpTf[:], in0=psq[:, 0:1], in1=bq[:, 0:1],
                            op=ALU.add)
    nc.vector.tensor_copy(QpTb[:], QpTf[:])
    # Block-diagonal Qhat[d, h] = Qp^T[d] * (d//32 == h)
    Qhat = sbuf.tile([128, H], bf16, tag="Qhat")
    nc.vector.tensor_tensor(out=Qhat[:], in0=_bcast(QpTb[:, 0:1], H),
                            in1=hmask[:], op=ALU.mult)
    return QpTf, Qhat


class _PmaLocal:
    """PMA computed from this core's local 1024 keys; the pair's partial
    A@V vectors are AllReduced (tiny [128,1] fp32) before fc_o."""

    def __init__(self, nc, pools, w, extras, QpTf, Qhat):
        self.nc, self.pools = nc, pools
        self.w, self.extras = w, extras
        self.QpTf, self.Qhat = QpTf, Qhat
        sbuf = pools["sbuf"]
        self.KT = sbuf.tile([128, NQ], bf16, tag="pKT")
        self.V = sbuf.tile([128, NQ], bf16, tag="pV")
        self.Ap = sbuf.tile([128, 32], bf16, tag="Ap")

    def wave(self, XhT, wv):
        nc, pools = self.nc, self.pools
        psP, psF = pools["psP"], pools["psF"]
        Wk, Wv = self.w["w2k"], self.w["w2v"]
        bk, bvb = self.w["b2k"], self.w["b2v"]
        cs = wv * QCH
        ps = psP.tile([128, QCH], fp32, tag="proj")
        nc.tensor.matmul(ps[:], lhsT=Wk[:], rhs=XhT[:, cs:cs + QCH],
                         start=True, stop=True)
        nc.vector.tensor_tensor(out=self.KT[:, cs:cs + QCH], in0=ps[:],
                                in1=_bcast(bk[:, 0:1], QCH), op=ALU.add)
        for tl in range(4):
            t = 4 * wv + tl
            ps = psP.tile([128, QCH], fp32, tag="proj")
            nc.tensor.matmul(ps[:, 0:128], lhsT=XhT[:, t * 128:(t + 1) * 128],
                             rhs=Wv[:], start=True, stop=True)
            nc.vector.tensor_tensor(out=self.V[:, t * 128:(t + 1) * 128],
                                    in0=ps[:, 0:128], in1=bvb[:], op=ALU.add)
        Lp = psF.tile([128, QCH], fp32, tag="F")
        for tl in range(4):
            t = 4 * wv + tl
            nc.tensor.matmul(Lp[:, 4 * tl:4 * tl + 4],
                             lhsT=self.KT[:, t * 128:(t + 1) * 128],
                             rhs=self.Qhat[:], start=True, stop=True)
        nc.scalar.activation(self.Ap[:, 16 * wv:16 * wv + 16], Lp[:, 0:16],
                             SIG, scale=float(SCALE))
        OpT_t = self.pools["psO"].tile([128, QCH], fp32, tag="OT")
        OpTps = OpT_t[:, 0:1]
        for h in range(H):
            for tl in range(4):
                t = 4 * wv + tl
                nc.tensor.matmul(
                    OpTps[32 * h:32 * h + 32, 0:1],
                    lhsT=self.V[:, 128 * t + 32 * h:128 * t + 32 * h + 32],
                    rhs=self.Ap[:, 4 * t + h:4 * t + h + 1],
                    start=(tl == 0), stop=(tl == 3),
                    tile_position=(0, 32 * h),
                    skip_group_check=True)
        # drain the per-wave partial and AllReduce it with the pair core
        # right away (wave-0's exchange hides under the rest of SAB1)
        dram = self.pools["dram"]
        AVw = self.pools["sbuf"].tile([128, 1], fp32, tag=f"AVw{wv}")
        nc.vector.tensor_copy(AVw[:], OpTps[:])
        cc_in = dram.tile([128, 1], fp32, tag=f"pcci{wv}")
        nc.gpsimd.dma_start(out=cc_in[:], in_=AVw[:])
        cc_out = dram.tile([128, 1], fp32, tag=f"pcco{wv}")
        nc.gpsimd.collective_compute(
            "AllReduce", ALU.add, replica_groups=PAIRS,
            ins=[cc_in[:]], outs=[cc_out[:]])
        ARw = self.pools["sbuf"].tile([128, 1], fp32, tag=f"ARw{wv}")
        nc.gpsimd.dma_start(out=ARw[:], in_=cc_out[:])
        if wv == 0:
            self.ARa = ARw
        else:
            self.ARb = ARw

    def finish(self):
        nc, pools = self.nc, self.pools
        sbuf, psO, psF, dram = (pools["sbuf"], pools["psO"], pools["psF"],
                                pools["dram"])
        Wo, bo = self.w["w2o"], self.w["b2o"]
        pW, pb = self.extras["pw"], self.extras["pb"]
        AVf = sbuf.tile([128, 1], fp32, tag="AVf")
        nc.vector.tensor_tensor(out=AVf[:], in0=self.ARa[:], in1=self.ARb[:],
                                op=ALU.add)
        OpTf = sbuf.tile([128, 1], fp32, tag="OpTf")
        OpTb = sbuf.tile([128, 1], bf16, tag="OpTb")
        nc.vector.tensor_tensor(out=OpTf[:], in0=AVf[:], in1=self.QpTf[:],
                                op=ALU.add)
        nc.vector.tensor_tensor(out=OpTb[:], in0=AVf[:], in1=self.QpTf[:],
                                op=ALU.add)
        FC2_t = psF.tile([128, QCH], fp32, tag="F")
        FC2 = FC2_t[:, 0:1]
        nc.tensor.matmul(FC2[:], lhsT=Wo[:], rhs=OpTb[:], start=True, stop=True)
        R2 = sbuf.tile([128, 1], fp32, tag="R2")
        nc.scalar.activation(R2[:], FC2[:], mybir.ActivationFunctionType.Relu,
                             bias=bo[:, 0:1])
        XpTb = sbuf.tile([128, 1], bf16, tag="XpTb")
        nc.vector.tensor_tensor(out=XpTb[:], in0=OpTf[:], in1=R2[:], op=ALU.add)
        OUT_t = psF.tile([128, QCH], fp32, tag="F")
        OUTps = OUT_t[0:1, 0:DOUT]
        nc.tensor.matmul(OUTps[:], lhsT=XpTb[:], rhs=pW[:], start=True,
                         stop=True)
        out_sb = sbuf.tile([1, DOUT], fp32, tag="out_sb")
        nc.vector.tensor_tensor(out=out_sb[:], in0=OUTps[:], in1=pb[:],
                                op=ALU.add)
        return out_sb


def build_program():
    nc = bass.Bass(num_devices=8)
    xt = nc.declare_dram_parameter("xt", [128, N], bf16, isOutput=False)
    xtq = nc.declare_dram_parameter("xtq", [128, NQ], bf16, isOutput=False)
    out_d = nc.declare_dram_parameter("out", [1, DOUT], fp32, isOutput=True)

    wshapes = {}
    for i in range(3):
        for k in ("q", "k", "v", "o"):
            wshapes[f"w{i}{k}"] = ([128, 128], bf16)
        wshapes[f"b{i}q"] = ([128, 1], fp32)
        wshapes[f"b{i}k"] = ([128, 1], fp32)
        wshapes[f"b{i}v"] = ([128, 128], fp32)  # pre-broadcast across partitions
        wshapes[f"b{i}o"] = ([128, 1], fp32)
    eshapes = {
        "st": ([128, 1], bf16),
        "hmask": ([128, H], bf16),
        "pw": ([128, DOUT], bf16),
        "pb": ([1, DOUT], fp32),
    }

    with tile.TileContext(nc) as tc:
        with (
            tc.tile_pool(name="sbuf", bufs=1) as sbuf,
            tc.tile_pool(name="sbufA", bufs=3) as sbufA,
            tc.tile_pool(name="psL", bufs=2, space="PSUM") as psL,
            tc.tile_pool(name="psO", bufs=1, space="PSUM") as psO,
            tc.tile_pool(name="psP", bufs=2, space="PSUM") as psP,
            tc.tile_pool(name="psF", bufs=1, space="PSUM") as psF,
            tc.tile_pool(name="dram", bufs=1, space="DRAM") as dram,
        ):
            pools = {"sbuf": sbuf, "sbufA": sbufA, "psL": psL, "psO": psO,
                     "psP": psP, "psF": psF, "dram": dram}

            # inputs: xt/xtq chunks on the HW-DGE queue, weights on SW-DGE
            XT0 = sbuf.tile([128, N], bf16, tag="XT0")
            XTq0 = sbuf.tile([128, NQ], bf16, tag="XTq0")
            nc.sync.dma_start(out=XTq0[:, 0:QCH], in_=xtq[:, 0:QCH])
            nc.sync.dma_start(out=XT0[:, 0:QCH], in_=xt[:, 0:QCH])
            nc.sync.dma_start(out=XTq0[:, QCH:NQ], in_=xtq[:, QCH:NQ])
            for c in range(1, 4):
                nc.sync.dma_start(out=XT0[:, c * QCH:(c + 1) * QCH],
                                  in_=xt[:, c * QCH:(c + 1) * QCH])
            w0 = {k: v for k, v in wshapes.items() if "0" in k}
            wrest = {k: v for k, v in wshapes.items() if "0" not in k}
            w = _load_weights(nc, sbuf, w0)
            w.update(_load_weights(nc, sbuf, wrest))
            extras = _load_weights(nc, sbuf, eshapes)
            # warm the ACT sigmoid table off the critical path
            warm = sbuf.tile([1, 1], fp32, tag="warm")
            nc.scalar.activation(warm[:], extras["pb"][0:1, 0:1], SIG)

            WAVES0 = [([0, 1], [0, 1, 2, 3, 4, 5, 6, 7]),
                      ([2, 3], [8, 9, 10, 11, 12, 13, 14, 15])]
            WAVES_AG = [([0, 2], [0, 1, 2, 3, 8, 9, 10, 11]),
                        ([1, 3], [4, 5, 6, 7, 12, 13, 14, 15])]
            Xh0, XT1 = _sab(nc, pools, XT0, XTq0, w, 0, "s0", WAVES0,
                            emit_ag=True)
            QpTf, Qhat = _pma_q(nc, pools, w, extras)
            pma = _PmaLocal(nc, pools, w, extras, QpTf, Qhat)
            post = {0: lambda XhT, qc: pma.wave(XhT, 0),
                    1: lambda XhT, qc: pma.wave(XhT, 1)}
            Xh1, _ = _sab(nc, pools, XT1, Xh0, w, 1, "s1", WAVES_AG,
                          emit_ag=False, post_qc=post)
            out_sb = pma.finish()
            nc.sync.dma_start(out=out_d[:], in_=out_sb[:])

    _fix_excess_waits(nc)
    return nc


_CACHE = {}


def _inputs_for_core(inputs, c):
    b, hf = c // 2, c % 2
    X = np.asarray(inputs["X"], dtype=np.float32)
    XT = np.ascontiguousarray(X[b].T).astype(ml_dtypes.bfloat16)
    m = {
        "xt": XT,
        "xtq": np.ascontiguousarray(XT[:, hf * NQ:(hf + 1) * NQ]),
        "st": np.ascontiguousarray(
            np.asarray(inputs["S"], np.float32).reshape(D, 1)
        ).astype(ml_dtypes.bfloat16),
        "hmask": (np.arange(128)[:, None] // 32 == np.arange(H)[None, :]
                  ).astype(ml_dtypes.bfloat16),
        "pw": np.ascontiguousarray(
            np.asarray(inputs["pW"], np.float32)).astype(ml_dtypes.bfloat16),
        "pb": np.asarray(inputs["pb"], np.float32).reshape(1, DOUT),
    }
    for i in range(3):
        for k in ("q", "k", "v", "o"):
            m[f"w{i}{k}"] = np.ascontiguousarray(
                np.asarray(inputs[f"m{i}_W{k}"], np.float32)
            ).astype(ml_dtypes.bfloat16)
        m[f"b{i}q"] = np.asarray(inputs[f"m{i}_bq"], np.float32).reshape(128, 1)
        m[f"b{i}k"] = np.asarray(inputs[f"m{i}_bk"], np.float32).reshape(128, 1)
        m[f"b{i}v"] = np.tile(
            np.asarray(inputs[f"m{i}_bv"], np.float32)[None, :], (128, 1))
        m[f"b{i}o"] = np.asarray(inputs[f"m{i}_bo"], np.float32).reshape(128, 1)
    return m


def kernel(**inputs) -> np.ndarray:
    if "nc" not in _CACHE:
        _CACHE["nc"] = build_program()
    nc = _CACHE["nc"]
    in_maps = [_inputs_for_core(inputs, c) for c in range(8)]
    res = run_bass_kernel_spmd(nc, in_maps, list(range(8)))
    out = np.stack([res.results[2 * b]["out"] for b in range(B)], axis=0)
    return out.astype(np.float32)  # [B, 1, DOUT]
